# revision 12
# baseline (speedup 1.0000x reference)
"""Trainium2 Bass kernel for nn_MultiHeadAttention (B=4, S=2048, D=1024, H=16, hd=64).

Sharding: 8 cores = 4 batches x 2 head-groups (DP on batch x TP on heads).
Each core computes, for its batch b and its 8 heads:
  qT/kT projections in transposed layout [d, s] (d on partitions),
  v projection in natural layout [s, d] augmented with a ones column per head
  (ones-row trick: the PV matmul then emits the softmax denominator for free),
  scoresT = kT.T @ qT per head in [s_k, s_q] layout (softmax-free-of-transposes),
  exp on ScalarE (no max subtraction: scores ~ N(0,1), exp is safe in fp32),
  PV in [d, s_q] layout, normalize by broadcast reciprocal of the denominator,
  out-projection partial [s, 1024].
Host sums the two head-group partials per batch and adds bo.

All heavy matmuls run in float32r (fp32 with mantissa rounded to 11 explicit
bits) which streams at 1 column/cycle on the PE vs 4 for exact fp32.
"""

import sys

sys.path.insert(0, "/opt/trn_rl_repo")

from contextlib import ExitStack

import numpy as np

import concourse.bass as bass  # noqa: F401  (import keeps bass registered)
import concourse.tile as tile
from concourse import bacc, mybir
from concourse.bass_utils import run_bass_kernel_spmd

B, S, D, H, HD = 4, 2048, 1024, 16, 64
HPC = 8  # heads per core
PAIRS = 4  # head pairs per core
DLOC = HPC * HD  # 512 local head dims
VAUG = HD + 2  # 66: head dim + ones col + pad col (fp32r needs even m)
F32 = mybir.dt.float32
F32R = mybir.dt.float32r
EXPF = mybir.ActivationFunctionType.Exp

NSQ = S // 512  # 4 s_q chunks
NSK = S // 128  # 16 s_k chunks
NFT = D // 128  # 8 feature tiles


def _round_fp32r(a: np.ndarray) -> np.ndarray:
    """Round fp32 to fp32r (drop low 12 mantissa bits, round half up)."""
    b = np.ascontiguousarray(a, dtype=np.float32).view(np.uint32)
    return ((b + 0x800) & 0xFFFFF000).astype(np.uint32).view(np.float32).reshape(a.shape)


def build(iters: int = 1):
    nc = bacc.Bacc("TRN2", target_bir_lowering=False, debug=False)
    xT = nc.dram_tensor("xT", [D, S], F32R, kind="ExternalInput").ap()
    wq = nc.dram_tensor("wq", [D, DLOC], F32R, kind="ExternalInput").ap()
    wk = nc.dram_tensor("wk", [D, DLOC], F32R, kind="ExternalInput").ap()
    wv = nc.dram_tensor("wv", [D, DLOC], F32R, kind="ExternalInput").ap()
    wo = nc.dram_tensor("wo", [DLOC, D], F32R, kind="ExternalInput").ap()
    bqk = nc.dram_tensor("bqk", [128, 8], F32, kind="ExternalInput").ap()
    bvd = nc.dram_tensor("bvd", [1, DLOC], F32, kind="ExternalInput").ap()
    # [1,0] pattern per head: fills the ones+pad columns of the augmented v tiles
    vones = nc.dram_tensor("vones", [128, 2 * HPC], F32R, kind="ExternalInput").ap()
    out = nc.dram_tensor("out", [S, D], F32, kind="ExternalOutput").ap()

    with tile.TileContext(nc) as tc, ExitStack() as ctx:
        persist = ctx.enter_context(tc.tile_pool(name="persist", bufs=1))
        for _ in range(iters):
            _body(nc, tc, persist, xT, wq, wk, wv, wo, bqk, bvd, vones, out)
    nc.compile()
    return nc


def _body(nc, tc, persist, xT, wq, wk, wv, wo, bqk, bvd, vones, out):
    # persistent tiles for this iteration
    q_t = [persist.tile([128, S], F32R, tag=f"q{p}", name=f"q{p}") for p in range(PAIRS)]
    k_t = [persist.tile([128, S], F32R, tag=f"k{p}", name=f"k{p}") for p in range(PAIRS)]
    v_t = [persist.tile([128, HPC * VAUG], F32R, tag=f"v{m}", name=f"v{m}") for m in range(NSK)]
    bqk_t = persist.tile([128, 8], F32, tag="bqk")
    nc.sync.dma_start(bqk_t[:], bqk[:])
    bv_row = persist.tile([1, DLOC], F32, tag="bvrow")
    nc.sync.dma_start(bv_row[:], bvd[:])
    bv_full = persist.tile([128, DLOC], F32, tag="bvfull")
    nc.gpsimd.partition_broadcast(bv_full[:], bv_row[:])

    # ---------------- projection phase ----------------
    # xT is processed in two S-halves (SBUF cannot hold all of xT alongside
    # q/k/v); each half covers s_q chunks 2h..2h+1 and s_k chunks 8h..8h+7.
    SH = S // 2
    with (
        tc.tile_pool(name="xt", bufs=1) as xtp,
        tc.tile_pool(name="wqk", bufs=6) as wqkp,
        tc.tile_pool(name="wvp", bufs=1) as wvp,
        tc.tile_pool(name="pproj", bufs=6, space="PSUM") as pproj,
    ):
        wv_t = []
        for ft in range(NFT):
            t = wvp.tile([128, DLOC], F32R, tag=f"wv{ft}")
            nc.sync.dma_start(t[:], wv[ft * 128 : (ft + 1) * 128, :])
            wv_t.append(t)

        for sh in range(2):
            xt = []
            for ft in range(NFT):
                t = xtp.tile([128, SH], F32R, tag=f"xt{ft}")
                nc.sync.dma_start(t[:], xT[ft * 128 : (ft + 1) * 128, sh * SH : (sh + 1) * SH])
                xt.append(t)

            # qT / kT projections: out [128 (pair dims), S-half]
            for p in range(PAIRS):
                for wdram, dst, bcol in ((wq, q_t[p], p), (wk, k_t[p], 4 + p)):
                    psums = [
                        pproj.tile([128, 512], F32, tag="pp", name="pp") for _ in range(2)
                    ]
                    for ft in range(NFT):
                        w_tile = wqkp.tile([128, 128], F32R, tag="w")
                        nc.sync.dma_start(
                            w_tile[:],
                            wdram[ft * 128 : (ft + 1) * 128, p * 128 : (p + 1) * 128],
                        )
                        for nl in range(2):
                            nc.tensor.matmul(
                                psums[nl][:],
                                w_tile[:],
                                xt[ft][:, nl * 512 : (nl + 1) * 512],
                                start=(ft == 0),
                                stop=(ft == NFT - 1),
                            )
                    for nl in range(2):
                        ncx = sh * 2 + nl
                        nc.vector.tensor_scalar_add(
                            dst[:, ncx * 512 : (ncx + 1) * 512],
                            psums[nl][:],
                            bqk_t[:, bcol : bcol + 1],
                        )

            # v projection (natural layout) + bias + ones/pad columns
            for ml in range(NSK // 2):
                mc = sh * (NSK // 2) + ml
                ps = pproj.tile([128, 512], F32, tag="pp")
                for ft in range(NFT):
                    nc.tensor.matmul(
                        ps[:],
                        xt[ft][:, ml * 128 : (ml + 1) * 128],
                        wv_t[ft][:],
                        start=(ft == 0),
                        stop=(ft == NFT - 1),
                    )
                dst3 = v_t[mc][:].rearrange("p (h c) -> p h c", c=VAUG)
                nc.vector.tensor_add(
                    dst3[:, :, 0:HD],
                    ps[:].rearrange("p (h c) -> p h c", c=HD),
                    bv_full[:].rearrange("p (h c) -> p h c", c=HD),
                )
                nc.sync.dma_start(
                    dst3[:, :, HD : HD + 2],
                    vones[:].rearrange("p (h c) -> p h c", c=2),
                )

    # ---------------- attention + output projection ----------------
    xa_t = [persist.tile([128, S], F32R, tag=f"xa{p}", name=f"xa{p}") for p in range(PAIRS)]
    with (
        tc.tile_pool(name="expp", bufs=3) as expp,
        tc.tile_pool(name="wop", bufs=2) as wop,
        tc.tile_pool(name="recp", bufs=2) as recp,
        tc.tile_pool(name="outp", bufs=2) as outp,
        tc.tile_pool(name="spool", bufs=2, space="PSUM") as spool,
        tc.tile_pool(name="pvp", bufs=4, space="PSUM") as pvp,
    ):
        for p in range(PAIRS):
            for sq in range(NSQ):
                pv_e = pvp.tile([128, 512], F32, tag="pv")
                pv_o = pvp.tile([128, 512], F32, tag="pv")
                for sk in range(NSK):
                    sc = spool.tile([128, 1024], F32, tag="sc")
                    # row-packed scores pair: head-even on PE rows 0-63,
                    # head-odd on rows 64-127 (concurrent quadrants)
                    nc.tensor.matmul(
                        sc[:, 0:512],
                        k_t[p][0:64, sk * 128 : (sk + 1) * 128],
                        q_t[p][0:64, sq * 512 : (sq + 1) * 512],
                    )
                    nc.tensor.matmul(
                        sc[:, 512:1024],
                        k_t[p][64:128, sk * 128 : (sk + 1) * 128],
                        q_t[p][64:128, sq * 512 : (sq + 1) * 512],
                    )
                    ex = expp.tile([128, 1024], F32R, tag="ex")
                    nc.scalar.activation(ex[:], sc[:], EXPF)
                    he, ho = 2 * p, 2 * p + 1
                    nc.tensor.matmul(
                        pv_e[0:VAUG, :],
                        v_t[sk][:, he * VAUG : he * VAUG + VAUG],
                        ex[:, 0:512],
                        start=(sk == 0),
                        stop=(sk == NSK - 1),
                    )
                    nc.tensor.matmul(
                        pv_o[0:VAUG, :],
                        v_t[sk][:, ho * VAUG : ho * VAUG + VAUG],
                        ex[:, 512:1024],
                        start=(sk == 0),
                        stop=(sk == NSK - 1),
                    )
                for hh, pv in ((0, pv_e), (1, pv_o)):
                    rec = recp.tile([1, 512], F32R, tag="rec")
                    with nc.allow_low_precision(
                        reason="fp32r keeps 12 mantissa bits; fine for softmax denom"
                    ):
                        nc.vector.reciprocal(rec[:], pv[HD : HD + 1, :])
                    bc = recp.tile([64, 512], F32R, tag="bc")
                    nc.gpsimd.partition_broadcast(bc[:], rec[:])
                    nc.vector.tensor_mul(
                        xa_t[p][hh * 64 : (hh + 1) * 64, sq * 512 : (sq + 1) * 512],
                        pv[0:64, :],
                        bc[:],
                    )

        # output projection: partial [S, D], one output-column half at a time
        # so only 4 Wo tiles are resident
        with tc.tile_pool(name="opool", bufs=2, space="PSUM") as opool:
            for nc2 in range(2):
                wo_t = []
                for kt in range(PAIRS):
                    t = wop.tile([128, 512], F32R, tag=f"wo{kt}", name=f"wo{kt}")
                    nc.sync.dma_start(
                        t[:], wo[kt * 128 : (kt + 1) * 128, nc2 * 512 : (nc2 + 1) * 512]
                    )
                    wo_t.append(t)
                for mc in range(NSK):
                    ps = opool.tile([128, 512], F32, tag="op")
                    for kt in range(PAIRS):
                        nc.tensor.matmul(
                            ps[:],
                            xa_t[kt][:, mc * 128 : (mc + 1) * 128],
                            wo_t[kt][:],
                            start=(kt == 0),
                            stop=(kt == PAIRS - 1),
                        )
                    st = outp.tile([128, 512], F32, tag="st")
                    nc.vector.tensor_copy(st[:], ps[:])
                    nc.sync.dma_start(
                        out[mc * 128 : (mc + 1) * 128, nc2 * 512 : (nc2 + 1) * 512], st[:]
                    )


_BUILT = {}


def _get_nc(iters: int = 1):
    if iters not in _BUILT:
        _BUILT[iters] = build(iters)
    return _BUILT[iters]


def make_in_maps(inputs_q, Wq, bq, Wk, bk, Wv, bv, Wo, bo):
    inputs_q = np.asarray(inputs_q, np.float32)
    Wq, bq = np.asarray(Wq, np.float32), np.asarray(bq, np.float32)
    Wk, bk = np.asarray(Wk, np.float32), np.asarray(bk, np.float32)
    Wv, bv = np.asarray(Wv, np.float32), np.asarray(bv, np.float32)
    Wo = np.asarray(Wo, np.float32)
    scale = 1.0 / np.sqrt(HD)
    in_maps = []
    for c in range(8):
        b, g = divmod(c, 2)
        hs = slice(g * HPC, (g + 1) * HPC)
        bqk_c = np.zeros((128, 8), np.float32)
        bq_c = (bq[hs].reshape(DLOC) * scale).astype(np.float32)
        bk_c = bk[hs].reshape(DLOC)
        for p in range(PAIRS):
            bqk_c[:, p] = bq_c[p * 128 : (p + 1) * 128]
            bqk_c[:, 4 + p] = bk_c[p * 128 : (p + 1) * 128]
        in_maps.append(
            {
                "xT": _round_fp32r(inputs_q[b].T),
                "wq": _round_fp32r(Wq[:, hs, :].reshape(D, DLOC) * scale),
                "wk": _round_fp32r(Wk[:, hs, :].reshape(D, DLOC)),
                "wv": _round_fp32r(Wv[:, hs, :].reshape(D, DLOC)),
                "wo": _round_fp32r(Wo[hs].reshape(DLOC, D)),
                "bqk": bqk_c,
                "bvd": bv[hs].reshape(1, DLOC).astype(np.float32),
                "vones": np.tile(np.array([1.0, 0.0], np.float32), (128, HPC)),
            }
        )
    return in_maps


def kernel(inputs_q, Wq, bq, Wk, bk, Wv, bv, Wo, bo, _iters: int = 1):
    nc = _get_nc(_iters)
    in_maps = make_in_maps(inputs_q, Wq, bq, Wk, bk, Wv, bv, Wo, bo)
    res = run_bass_kernel_spmd(nc, in_maps, list(range(8))).results
    bo = np.asarray(bo, np.float32)
    out = np.empty((B, S, D), np.float32)
    for b in range(B):
        out[b] = res[2 * b]["out"] + res[2 * b + 1]["out"] + bo[None, :]
    return out
